# revision 21
# baseline (speedup 1.0000x reference)
"""Trainium2 Bass kernel for nn_ClassModel_72318659330833.

Strategy:
  - Data-parallel over batch: 8 cores x 32 rows.
  - Host-side prep (cheap, index/layout only): find mask positions, gather
    mask hidden states + pooler rows, build additive softmax mask, transpose
    weights into lhsT layouts, slice the 30522-row decoder down to the 21
    label rows actually used by the output.
  - Device per core: attention scores via PE (f32, d-on-partition X^T),
    masked softmax (DVE+ACT), attention-weighted sum via PE (bf16, s-on-
    partition X), dense tanh layer via PE (bf16, output directly in
    transposed layout), 21-row decoder + label-word mixing + sentiment head.
"""

import numpy as np
import ml_dtypes

import concourse.bacc as bacc
import concourse.bass as bass
import concourse.tile as tile
from concourse import mybir
from concourse import bass_utils
from concourse.masks import make_identity

BF16 = ml_dtypes.bfloat16

B, S, D, V = 256, 128, 768, 30522
MASK_ID = 103
NCORES = 8
BC = B // NCORES          # 32 rows per core
L = 104                   # max attention window length (length < 105)
C6 = D // 128             # 6 chunks of 128 along D
C12 = 2 * D // 128        # 12 chunks along 2D

LABEL_IDS = [
    [2307, 2204, 3835, 2157, 6581, 2986, 5151, 3893],
    [7929, 24791, 8699, 4257, 16021, 6623],
    [6659, 2919, 11771, 3532, 11325, 4997, 13135],
]
FLAT_IDS = [i for g in LABEL_IDS for i in g]  # 21 ids
NK = len(FLAT_IDS)

KERNEL_TRACE = False      # test.py sets True to capture NTFF exec time
LAST_RESULT = None        # bass_utils.BassKernelResults of last run

_compiled = None


def _body(nc, tc, t):
    """Emit the per-core kernel body. t: dict name->dram AP."""
    f32 = mybir.dt.float32
    bf16 = mybir.dt.bfloat16

    from contextlib import ExitStack

    ctx = ExitStack()
    singles = ctx.enter_context(tc.tile_pool(name="singles", bufs=1))
    work = ctx.enter_context(tc.tile_pool(name="work", bufs=1))
    psum = ctx.enter_context(tc.tile_pool(name="psum", bufs=1, space="PSUM"))

    # ---- resident SBUF tiles, one DMA each -------------------------------
    def load(name, shape, dt):
        s = singles.tile(shape, dt, tag=name)
        nc.sync.dma_start(out=s[:], in_=t[name][:])
        return s

    x_sb = load("x", [L, BC, D], bf16)            # X natural  [s, b, d]
    xt_sb = load("xt", [128, C6, BC * L], f32)    # X^T chunks [dd, c, (b s)]
    mt_sb = load("mt", [128, C6, BC], f32)        # m^T chunks (f32, scores)
    mtb_sb = load("mtb", [128, C6, BC], bf16)     # m^T chunks (bf16, dense rhs)
    poolert_sb = load("poolert", [128, C6, BC], f32)
    wdt_sb = load("wdt", [128, C12, D], bf16)     # dense_w^T chunks
    dect_sb = load("dect", [128, C6, NK], bf16)   # 21-row decoder^T chunks
    sentit_sb = load("sentit", [128, C6, 2], f32)
    wf_sb = load("wf", [NK, 6], f32)
    db_sb = load("db", [128, C6], f32)            # dense_b chunks
    dbdec_sb = load("dbdec", [NK, 1], f32)
    sb_sb = load("sb", [2, 1], f32)
    amask_sb = load("amask", [BC, L], f32)        # 0 valid / -1e30 invalid
    dmask_sb = load("dmask", [BC, BC // 8, 8, L], f32)  # diag-block selector

    ident_f = singles.tile([BC, BC], f32, tag="identf")
    make_identity(nc, ident_f[:])

    # ---- 1+2. attention scores via all-pairs matmuls ---------------------
    # S_all[b', (b,s)] = m_b' . X_b[s]; diagonal blocks b'==b are the real
    # scores. Processed in 4 quarters of 8 rows (2 PSUM banks each, double
    # buffered) with per-row diagonal extraction on DVE.
    QB = 8                                        # b-rows per quarter
    NQ = BC // QB                                 # 4 quarters
    QN = QB * L                                   # 832 free elems
    reds = []
    for q in range(NQ):
        ps_q = psum.tile([BC, QN], f32, tag="big", bufs=2)
        for (j0, jn) in ((0, 512), (512, QN - 512)):
            for c in range(C6):
                nc.tensor.matmul(
                    ps_q[:, j0 : j0 + jn],
                    mt_sb[:, c, :],              # lhsT [128, 32]
                    xt_sb[:, c, q * QN + j0 : q * QN + j0 + jn],
                    start=(c == 0),
                    stop=(c == C6 - 1),
                )
        # zero out the off-diagonal b-blocks, then reduce over the block idx
        masked = work.tile([BC, QB, L], f32, tag="masked", bufs=2)
        nc.vector.tensor_tensor(
            out=masked[:],
            in0=ps_q[:].rearrange("p (i s) -> p i s", i=QB),
            in1=dmask_sb[:, q, :, :],
            op=mybir.AluOpType.mult,
        )
        red = work.tile([BC, L], f32, tag="red", bufs=NQ)
        nc.vector.tensor_reduce(
            out=red[:],
            in_=masked[:].rearrange("p i s -> p s i"),
            axis=mybir.AxisListType.X,
            op=mybir.AluOpType.add,
        )
        reds.append(red)
    scores_sb = work.tile([BC, L], f32, tag="scores_sb")
    nc.vector.tensor_tensor(
        out=scores_sb[:], in0=reds[0][:], in1=reds[1][:], op=mybir.AluOpType.add
    )
    nc.vector.tensor_tensor(
        out=reds[2][:], in0=reds[2][:], in1=reds[3][:], op=mybir.AluOpType.add
    )
    nc.vector.tensor_tensor(
        out=scores_sb[:], in0=scores_sb[:], in1=reds[2][:], op=mybir.AluOpType.add
    )
    nc.vector.tensor_tensor(
        out=scores_sb[:], in0=scores_sb[:], in1=amask_sb[:], op=mybir.AluOpType.add
    )
    mx = work.tile([BC, 1], f32, tag="mx")
    nc.vector.tensor_reduce(
        out=mx[:], in_=scores_sb[:], axis=mybir.AxisListType.X, op=mybir.AluOpType.max
    )
    negmax = work.tile([BC, 1], f32, tag="negmax")
    nc.vector.tensor_scalar_mul(negmax[:], mx[:], -1.0)
    p_sb = work.tile([BC, L], f32, tag="p_sb")
    sumexp = work.tile([BC, 1], f32, tag="sumexp")
    nc.scalar.activation(
        out=p_sb[:],
        in_=scores_sb[:],
        func=mybir.ActivationFunctionType.Exp,
        bias=negmax[:],
        scale=1.0,
        accum_out=sumexp[:],
    )
    rsum = work.tile([BC, 1], f32, tag="rsum")
    nc.vector.reciprocal(out=rsum[:], in_=sumexp[:])
    nc.vector.tensor_scalar_mul(p_sb[:], p_sb[:], rsum[:])

    # ---- 3. p^T  (transpose [BC, L] -> [L, BC], cast bf16) ---------------
    ps_pt = psum.tile([L, BC], f32, tag="small")
    nc.tensor.transpose(ps_pt[:], p_sb[:], ident_f[:])
    pt_sb = work.tile([L, BC], bf16, tag="pt_sb")
    nc.vector.tensor_copy(pt_sb[:], ps_pt[:])

    # ---- 4+5. att^T[d, b] = sum_s X[b,s,d] p[b,s], built directly in the
    # transposed layout: per (b, d-chunk) matmul with X slice stationary.
    ps_attt = psum.tile([128, C6, BC], f32, tag="attt")
    for b in range(BC):
        for c in range(C6):
            nc.tensor.matmul(
                ps_attt[:, c, b : b + 1],
                x_sb[:, b, c * 128 : (c + 1) * 128],  # lhsT [L, 128]
                pt_sb[:, b : b + 1],                  # rhs  [L, 1]
                start=True,
                stop=True,
            )
    attt_sb = work.tile([128, C6, BC], bf16, tag="attt_sb")
    nc.vector.tensor_copy(attt_sb[:], ps_attt[:])

    # ---- 6. dense: h^T[o, b] = tanh(sum_i W[o,i] feats[b,i] + db[o]) -----
    ps_ht = psum.tile([128, C6, BC], f32, tag="ht")
    for c in range(C6):
        for k in range(C12):
            rhs = attt_sb[:, k, :] if k < C6 else mtb_sb[:, k - C6, :]
            nc.tensor.matmul(
                ps_ht[:, c, :],
                wdt_sb[:, k, c * 128 : (c + 1) * 128],  # lhsT [128, 128]
                rhs,                                     # rhs  [128, BC]
                start=(k == 0),
                stop=(k == C12 - 1),
            )
    ht_sb = work.tile([128, C6, BC], bf16, tag="ht_sb")
    for c in range(C6):
        nc.scalar.activation(
            out=ht_sb[:, c, :],
            in_=ps_ht[:, c, :],
            func=mybir.ActivationFunctionType.Tanh,
            bias=db_sb[:, c : c + 1],
            scale=1.0,
        )

    # ---- 7. decoder (21 label rows): p21^T[k, b] -------------------------
    ps_p21 = psum.tile([NK, BC], f32, tag="small")
    for c in range(C6):
        nc.tensor.matmul(
            ps_p21[:],
            dect_sb[:, c, :],                    # lhsT [128, 21]
            ht_sb[:, c, :],                      # rhs  [128, BC]
            start=(c == 0),
            stop=(c == C6 - 1),
        )
    p21t_sb = work.tile([NK, BC], f32, tag="p21t_sb")
    nc.scalar.activation(
        out=p21t_sb[:],
        in_=ps_p21[:],
        func=mybir.ActivationFunctionType.Tanh,
        bias=dbdec_sb[:],
        scale=1.0,
    )

    # ---- 8. label mixing: out6[b, j*3+g] ---------------------------------
    ps_out6 = psum.tile([BC, 6], f32, tag="small")
    nc.tensor.matmul(ps_out6[:], p21t_sb[:], wf_sb[:], start=True, stop=True)
    out6_sb = work.tile([BC, 6], f32, tag="out6_sb")
    nc.vector.tensor_copy(out6_sb[:], ps_out6[:])
    nc.sync.dma_start(out=t["out6"][:], in_=out6_sb[:])

    # ---- 9. sentiment head: cat^T[c, b] ----------------------------------
    ps_cat = psum.tile([2, BC], f32, tag="small")
    for c in range(C6):
        nc.tensor.matmul(
            ps_cat[:],
            sentit_sb[:, c, :],                  # lhsT [128, 2]
            poolert_sb[:, c, :],                 # rhs  [128, BC]
            start=(c == 0),
            stop=(c == C6 - 1),
        )
    catt_sb = work.tile([2, BC], f32, tag="catt_sb")
    nc.vector.tensor_scalar_add(catt_sb[:], ps_cat[:], sb_sb[:])
    nc.sync.dma_start(out=t["catt"][:], in_=catt_sb[:])

    ctx.close()


def _build():
    global _compiled
    if _compiled is not None:
        return _compiled
    f32 = mybir.dt.float32
    bf16 = mybir.dt.bfloat16
    nc = bacc.Bacc("TRN2", target_bir_lowering=False, debug=False)
    t = {}

    def din(name, shape, dt):
        t[name] = nc.dram_tensor(name, shape, dt, kind="ExternalInput").ap()

    def dout(name, shape, dt):
        t[name] = nc.dram_tensor(name, shape, dt, kind="ExternalOutput").ap()

    din("x", [L, BC, D], bf16)
    din("xt", [128, C6, BC * L], f32)
    din("mt", [128, C6, BC], f32)
    din("mtb", [128, C6, BC], bf16)
    din("poolert", [128, C6, BC], f32)
    din("wdt", [128, C12, D], bf16)
    din("dect", [128, C6, NK], bf16)
    din("sentit", [128, C6, 2], f32)
    din("wf", [NK, 6], f32)
    din("db", [128, C6], f32)
    din("dbdec", [NK, 1], f32)
    din("sb", [2, 1], f32)
    din("amask", [BC, L], f32)
    din("dmask", [BC, BC // 8, 8, L], f32)
    dout("out6", [BC, 6], f32)
    dout("catt", [2, BC], f32)

    with tile.TileContext(nc) as tc:
        _body(nc, tc, t)
    nc.compile()
    _compiled = nc
    return nc


def _chunkT(a2d):
    """[N, D-like] -> [128, D//128, N] chunked transpose layout."""
    d = a2d.shape[1]
    return np.ascontiguousarray(a2d.T.reshape(d // 128, 128, a2d.shape[0]).transpose(1, 0, 2))


def kernel(**inputs):
    global LAST_RESULT
    bert = np.asarray(inputs["bert_out"], dtype=np.float32)      # [B, S, D]
    ids = np.asarray(inputs["input_ids"])
    length = np.asarray(inputs["length"]).astype(np.int64)
    senti_w = np.asarray(inputs["senti_w"], dtype=np.float32)
    senti_b = np.asarray(inputs["senti_b"], dtype=np.float32)
    dense_w = np.asarray(inputs["dense_w"], dtype=np.float32)    # [D, 2D]
    dense_b = np.asarray(inputs["dense_b"], dtype=np.float32)
    dec_w = np.asarray(inputs["dec_w"], dtype=np.float32)        # [V, D]
    dec_b = np.asarray(inputs["dec_b"], dtype=np.float32)
    w0 = np.asarray(inputs["w0"], dtype=np.float32)
    w1 = np.asarray(inputs["w1"], dtype=np.float32)
    w2 = np.asarray(inputs["w2"], dtype=np.float32)

    mask_pos = np.argmax(ids == MASK_ID, axis=1)                 # [B]
    m = bert[np.arange(B), mask_pos]                             # [B, D]
    pooler = bert[:, 0]                                          # [B, D]
    xs = bert[:, 3 : 3 + L]                                      # [B, L, D]

    # shared weight layouts
    wdt = np.ascontiguousarray(
        dense_w.T.reshape(C12, 128, D).transpose(1, 0, 2)
    ).astype(BF16)                                               # [128, 12, D]
    dec21 = dec_w[FLAT_IDS]                                      # [21, D]
    dect = _chunkT(dec21).astype(BF16)                           # [128, 6, 21]
    sentit = _chunkT(senti_w)                                    # [128, 6, 2]
    wf = np.zeros((NK, 6), np.float32)
    off = 0
    for g, wg in enumerate((w0, w1, w2)):
        k = wg.shape[1]
        for j in range(2):
            wf[off : off + k, j * 3 + g] = wg[j]
        off += k
    db = np.ascontiguousarray(dense_b.reshape(C6, 128).T)        # [128, 6]
    dmask = np.zeros((BC, BC // 8, 8, L), np.float32)
    for b in range(BC):
        dmask[b, b // 8, b % 8, :] = 1.0
    dbdec = dec_b[FLAT_IDS][:, None].astype(np.float32)
    sb = senti_b[:, None]

    nc = _build()

    in_maps = []
    for k in range(NCORES):
        sl = slice(k * BC, (k + 1) * BC)
        xsk = xs[sl]                                             # [BC, L, D]
        x_in = np.ascontiguousarray(xsk.transpose(1, 0, 2)).astype(BF16)
        xt_in = np.ascontiguousarray(
            xsk.transpose(2, 0, 1).reshape(C6, 128, BC, L).transpose(1, 0, 2, 3)
        ).reshape(128, C6, BC * L)                               # [128, 6, 3328] f32
        mtk = _chunkT(m[sl])                                     # [128, 6, BC]
        amask = np.where(
            np.arange(L)[None, :] < length[sl, None], 0.0, -1e30
        ).astype(np.float32)
        in_maps.append(
            {
                "x": x_in,
                "xt": xt_in,
                "mt": mtk,
                "mtb": mtk.astype(BF16),
                "poolert": _chunkT(pooler[sl]),
                "wdt": wdt,
                "dect": dect,
                "sentit": sentit,
                "wf": wf,
                "db": db,
                "dbdec": dbdec,
                "sb": sb,
                "amask": amask,
                "dmask": dmask,
            }
        )

    res = bass_utils.run_bass_kernel_spmd(
        nc, in_maps, core_ids=list(range(NCORES)), trace=KERNEL_TRACE
    )
    LAST_RESULT = res

    category_out = np.empty((B, 2), np.float32)
    out = np.empty((B, 2, 3), np.float32)
    for k in range(NCORES):
        sl = slice(k * BC, (k + 1) * BC)
        category_out[sl] = res.results[k]["catt"].T
        out[sl] = res.results[k]["out6"].reshape(BC, 2, 3)
    return category_out, out
